# revision 11
# baseline (speedup 1.0000x reference)
"""Trainium2 Bass kernel for DEIM multi-scale deformable attention (v2).

Strategy:
  - Data-parallel over batch: 16 batches -> 8 cores, 2 batches/core.
  - Host pre-packs memory as bf16 "rows4" im2col: rows4[(b,l,y,x)] holds
    pixel rows y..y+3 of column x (4 x 256 ch = 2KB), so one 4x4-pixel
    window x 256 ch is ONE contiguous 8KB dma_gather descriptor
    (order j=x-offset, r=y-offset, c).  Window origins, gather indices
    (pre-wrapped for dma_gather) and pxm = refpix - origin are all
    computed on the host - no on-device geometry or index bounce.
  - Device per query-tile (queries on partitions, 5 tiles of <=128):
    PE projections -> softmax -> bilinear hat factors ->
    stencil M[q,(h,j,r)] = sum_p attn*hatx*haty -> ACT broadcasts it to
    a bf16 meexp[q,(j,r,c)] -> DVE: one 2x-mode bf16 multiply with the
    gathered window + in-place bf16 add-tree over the 16 pixels ->
    f32 level sum -> PE output projection.
"""

import os
from contextlib import ExitStack

import numpy as np

# ---------------------------------------------------------------------------
# Problem constants (hardcoded per harness contract)
# ---------------------------------------------------------------------------
B, Q, C, NH, NP, NL = 16, 300, 256, 8, 4, 4
HD = C // NH
SPATIAL = ((80, 80), (40, 40), (20, 20), (30, 70))  # (h, w) per level
S = sum(h * w for h, w in SPATIAL)  # 10500
H_L = [h for h, w in SPATIAL]
W_L = [w for h, w in SPATIAL]

NCORES = 8
BPC = B // NCORES          # batches per core
QS = BPC * Q               # query slots per core (600)
QT_SIZES = [128, 128, 128, 128, QS - 4 * 128]  # [128,128,128,128,88]
NQT = len(QT_SIZES)
WIN = 4                    # window size (pixels per axis)
WELEM = WIN * WIN * C      # window elements (4096 bf16 = 8KB)

# rows4 geometry: per (batch, level) block of (h-3)*w start slots
R4_L = [(h - 3) * w for h, w in SPATIAL]          # [6160, 1480, 340, 1890]
R4_B = sum(R4_L)                                   # 9870 slots per batch
R4BASE = [0]
for v in R4_L[:-1]:
    R4BASE.append(R4BASE[-1] + v)                  # [0, 6160, 7640, 7980]
NR4 = BPC * R4_B                                   # 19740 slots per core


def _build_program():
    import concourse.bacc as bacc
    import concourse.bass as bass
    import concourse.tile as tile
    from concourse import mybir
    from concourse.masks import make_identity

    f32 = mybir.dt.float32
    bf16 = mybir.dt.bfloat16
    i16 = mybir.dt.int16

    nc = bacc.Bacc("TRN2", target_bir_lowering=False, debug=False,
                   num_devices=NCORES)

    AF = mybir.ActivationFunctionType
    OP = mybir.AluOpType

    def ap_of(t, off, pairs):
        """Manual access pattern on a tile/AP: offset in elements relative
        to t's own offset; pairs = [[step, count], ...] (partition first,
        rescaled to the tensor's per-partition stride; free steps in
        elements)."""
        a = t[:] if hasattr(t, "__getitem__") else t
        pairs = [list(p) for p in pairs]
        if a.space == bass.MemorySpace.SBUF:
            pairs[0][0] *= a.ap[0][0]
        return bass.AP(tensor=a.tensor, offset=a.offset + off, ap=pairs)

    # ------------------------------------------------------------------
    # DRAM I/O
    # ------------------------------------------------------------------
    mem4d = nc.dram_tensor("mem4", [NR4, WIN * C], bf16, kind="ExternalInput")
    qTd = nc.dram_tensor("qT", [C, QS], f32, kind="ExternalInput")
    idxd = nc.dram_tensor("idxw", [128, NQT * 32], i16, kind="ExternalInput")
    pxmd = nc.dram_tensor("pxm", [QS, 2 * NL], f32, kind="ExternalInput")
    woffd = nc.dram_tensor("Woff", [C, 256], f32, kind="ExternalInput")
    wattnd = nc.dram_tensor("Wattn", [C, NH * NL * NP], f32, kind="ExternalInput")
    woutd = nc.dram_tensor("Wout", [C, C], f32, kind="ExternalInput")
    boutd = nc.dram_tensor("bout", [1, C], f32, kind="ExternalInput")
    outd = nc.dram_tensor("out", [QS, C], f32, kind="ExternalOutput")

    with tile.TileContext(nc) as tc, ExitStack() as ctx:
        singles = ctx.enter_context(tc.tile_pool(name="singles", bufs=1))
        psum_mm = ctx.enter_context(tc.tile_pool(name="psum_mm", bufs=2, space="PSUM"))
        psum_tr = ctx.enter_context(tc.tile_pool(name="psum_tr", bufs=2, space="PSUM"))
        psum_o = ctx.enter_context(tc.tile_pool(name="psum_o", bufs=2, space="PSUM"))
        work = ctx.enter_context(tc.tile_pool(name="work", bufs=2))
        winp = ctx.enter_context(tc.tile_pool(name="winp", bufs=2))
        mep = ctx.enter_context(tc.tile_pool(name="mep", bufs=2))

        # ---------------- one-time constants ----------------
        sb_qT = singles.tile([128, 2, QS], f32)
        nc.sync.dma_start(out=sb_qT, in_=qTd.ap().rearrange("(k p) q -> p k q", p=128))
        sb_Woff = singles.tile([128, 2, 256], f32)
        nc.sync.dma_start(out=sb_Woff, in_=woffd.ap().rearrange("(k p) n -> p k n", p=128))
        sb_Wattn = singles.tile([128, 2, 128], f32)
        nc.sync.dma_start(out=sb_Wattn, in_=wattnd.ap().rearrange("(k p) n -> p k n", p=128))
        sb_Wout = singles.tile([128, 2, 256], f32)
        nc.sync.dma_start(out=sb_Wout, in_=woutd.ap().rearrange("(k p) n -> p k n", p=128))
        sb_bout = singles.tile([1, 256], f32)
        nc.sync.dma_start(out=sb_bout, in_=boutd.ap())
        sb_ones = singles.tile([1, 128], f32)
        nc.vector.memset(sb_ones, 1.0)
        ident = singles.tile([128, 128], f32)
        make_identity(nc, ident[:])
        # all gather indices, pre-wrapped by the host: [128, t, 32]
        sb_idx = singles.tile([128, NQT, 32], i16)
        nc.sync.dma_start(out=sb_idx, in_=idxd.ap())
        jneg = singles.tile([128, WIN], f32)
        for j in range(WIN):
            nc.vector.memset(jneg[:, j:j + 1], float(-j))

        # ---------------- per query-tile pipeline ----------------
        for it in range(NQT):
            q0 = it * 128
            qlen = QT_SIZES[it]
            ql = slice(0, qlen)

            # --- PE projections: offs [q, (l,h,p,xy)], logits [q, (h,l,p)]
            ps_off = psum_mm.tile([128, 256], f32, tag="ps_off")
            nc.tensor.matmul(ps_off[ql, :], lhsT=sb_qT[:, 0, q0:q0 + qlen],
                             rhs=sb_Woff[:, 0, :], start=True, stop=False)
            nc.tensor.matmul(ps_off[ql, :], lhsT=sb_qT[:, 1, q0:q0 + qlen],
                             rhs=sb_Woff[:, 1, :], start=False, stop=True)
            ps_log = psum_mm.tile([128, 128], f32, tag="ps_log")
            nc.tensor.matmul(ps_log[ql, :], lhsT=sb_qT[:, 0, q0:q0 + qlen],
                             rhs=sb_Wattn[:, 0, :], start=True, stop=False)
            nc.tensor.matmul(ps_log[ql, :], lhsT=sb_qT[:, 1, q0:q0 + qlen],
                             rhs=sb_Wattn[:, 1, :], start=False, stop=True)

            offs = work.tile([128, 256], f32, tag="offs")
            nc.scalar.copy(offs[ql, :], ps_off[ql, :])

            # --- softmax over (l,p) per h; logits cols are (h,l,p)
            elog = work.tile([128, 128], f32, tag="elog")
            nc.scalar.activation(elog[ql, :], ps_log[ql, :], AF.Exp)
            ssum = work.tile([128, NH], f32, tag="ssum")
            nc.vector.tensor_reduce(ssum[ql, :],
                                    elog[ql, :].rearrange("q (h s) -> q h s", h=NH),
                                    axis=mybir.AxisListType.X, op=OP.add)
            rinv = work.tile([128, NH], f32, tag="rinv")
            nc.vector.reciprocal(rinv[ql, :], ssum[ql, :])
            # attnR[q, (l,h,p)] = elog[q, h,l,p] * rinv[q, h]
            attnR = work.tile([128, 128], f32, tag="attnR")
            nc.vector.tensor_mul(
                attnR[ql, :],
                ap_of(elog, 0, [[1, qlen], [4, NL], [16, NH], [1, NP]]),
                ap_of(rinv, 0, [[1, qlen], [0, NL], [1, NH], [0, NP]]),
            )

            # --- window-relative positions pxm (host-computed) [q, (l,xy)]
            pxm = work.tile([128, 2 * NL], f32, tag="pxm")
            nc.sync.dma_start(out=pxm[ql, :], in_=pxmd.ap()[q0:q0 + qlen, :])

            # --- hats: U[q, (l,xy,hp)] = offs + pxm ;  H = relu(1 - |U - j|)
            uu = work.tile([128, NL, 2, 32], f32, tag="uu")
            for l in range(NL):
                for xy in range(2):
                    nc.scalar.activation(
                        uu[ql, l, xy, :],
                        ap_of(offs, l * 64 + xy, [[1, qlen], [2, 32]]),
                        AF.Identity,
                        bias=pxm[ql, 2 * l + xy:2 * l + xy + 1], scale=1.0)
            hat = work.tile([128, WIN, NL, 2, 32], f32, tag="hat")
            for j in range(WIN):
                nc.scalar.activation(hat[ql, j, :, :, :],
                                     uu[ql, :, :, :], AF.Abs,
                                     bias=jneg[ql, j:j + 1])
            nc.scalar.activation(hat[ql, :, :, :, :], hat[ql, :, :, :, :],
                                 AF.Relu, bias=1.0, scale=-1.0)

            # AFJ[q, (l,h,p,j)] = attnR[q,(l,h,p)] * hatx[q,(j,l,hp)]
            afj = work.tile([128, NL, 8, NP, WIN], f32, tag="afj")
            nc.vector.tensor_mul(
                afj[ql, :, :, :, :],
                ap_of(hat, 0, [[1, qlen], [64, NL], [1, 32], [256, WIN]]),
                ap_of(attnR, 0, [[1, qlen], [32, NL], [1, 32], [0, WIN]]))

            # --- one gather per tile: all 4 levels, idx k = l*128 + q
            win = winp.tile([128, NL, WELEM], bf16, tag="win")
            nc.gpsimd.dma_gather(
                out_ap=win[:, :, :],
                in_ap=ap_of(mem4d.ap(), 0, [[WIN * C, NR4 - (WIN - 1)], [1, WELEM]]),
                idxs_ap=sb_idx[:, it, :],
                num_idxs=512, num_idxs_reg=512,
                elem_size=WELEM, elem_step=WIN * C)

            # --- stencil per level (GpSimd): prod, mm; me3a slice (DVE)
            me3a = work.tile([128, NL, 128], f32, tag="me3a")
            for l in range(NL):
                # prod[q, (h,j,r), p] = afj[q,(l,h,p,j)] * haty[q,(r,l,hp)]
                prod = work.tile([128, 8 * WIN * WIN, NP], f32, tag="prod")
                for p in range(NP):
                    nc.gpsimd.tensor_mul(
                        ap_of(prod, p, [[1, qlen], [NP, 8 * WIN * WIN]]),
                        ap_of(afj, l * 128 + p * WIN,
                              [[1, qlen], [16, 8], [1, WIN], [0, WIN]]),
                        ap_of(hat, l * 64 + 32 + p,
                              [[1, qlen], [4, 8], [0, WIN], [256, WIN]]))
                # me3a[q, l, (j,r,h)] = sum_p prod[q, (h,j,r), p]
                nc.vector.tensor_reduce(
                    ap_of(me3a, l * 128, [[1, qlen], [1, 8], [32, WIN], [8, WIN]]),
                    prod[ql, :, :],
                    axis=mybir.AxisListType.X, op=OP.add)

            # --- meexp[q, l, (j,r,h)*32+rep]: ACT does levels 0-2, GpSimd 3
            meexp = mep.tile([128, NL, WELEM], bf16, tag="meexp")
            nc.scalar.activation(
                ap_of(meexp, 0, [[1, qlen], [32, 384], [1, 32]]),
                ap_of(me3a, 0, [[1, qlen], [1, 384], [0, 32]]),
                AF.Identity)
            nc.gpsimd.tensor_copy(
                ap_of(meexp, 3 * WELEM, [[1, qlen], [32, 128], [1, 32]]),
                ap_of(me3a, 3 * 128, [[1, qlen], [1, 128], [0, 32]]))

            # --- prodw = win * meexp in place (bf16 unit stride -> 2x mode)
            nc.vector.tensor_mul(
                ap_of(win, 0, [[1, qlen], [1, NL * WELEM]]),
                ap_of(win, 0, [[1, qlen], [1, NL * WELEM]]),
                ap_of(meexp, 0, [[1, qlen], [1, NL * WELEM]]))
            # --- in-place bf16 add-tree over the 16 pixels, all levels
            for half in (2048, 1024, 512):
                nc.vector.tensor_add(
                    ap_of(win, 0, [[1, qlen], [WELEM, NL], [1, half]]),
                    ap_of(win, 0, [[1, qlen], [WELEM, NL], [1, half]]),
                    ap_of(win, half, [[1, qlen], [WELEM, NL], [1, half]]))
            res4b = work.tile([128, NL, 256], bf16, tag="res4b")
            nc.vector.tensor_add(
                res4b[ql, :, :],
                ap_of(win, 0, [[1, qlen], [WELEM, NL], [1, 256]]),
                ap_of(win, 256, [[1, qlen], [WELEM, NL], [1, 256]]))
            # --- level sum: bf16 then f32 out
            nc.vector.tensor_add(
                ap_of(res4b, 0, [[1, qlen], [1, 512]]),
                ap_of(res4b, 0, [[1, qlen], [1, 512]]),
                ap_of(res4b, 512, [[1, qlen], [1, 512]]))
            res = work.tile([128, 256], f32, tag="res")
            nc.vector.tensor_add(res[ql, :],
                                 ap_of(res4b, 0, [[1, qlen], [1, 256]]),
                                 ap_of(res4b, 256, [[1, qlen], [1, 256]]))

            # --- output projection: out = res @ Wout + bout
            resT = work.tile([128, 2, 128], f32, tag="resT")
            for hh in range(2):
                ps_t = psum_tr.tile([128, 128], f32, tag="ps_t")
                nc.tensor.transpose(ps_t[:, ql], res[ql, 128 * hh:128 * (hh + 1)],
                                    ident[ql, ql])
                nc.scalar.copy(resT[:, hh, ql], ps_t[:, ql])
            ps_out = psum_o.tile([128, 256], f32, tag="ps_out")
            nc.tensor.matmul(ps_out[ql, :], lhsT=resT[:, 0, ql],
                             rhs=sb_Wout[:, 0, :], start=True, stop=False)
            nc.tensor.matmul(ps_out[ql, :], lhsT=resT[:, 1, ql],
                             rhs=sb_Wout[:, 1, :], start=False, stop=False)
            nc.tensor.matmul(ps_out[ql, :], lhsT=sb_ones[0:1, ql],
                             rhs=sb_bout[0:1, :], start=False, stop=True)
            outt = work.tile([128, 256], f32, tag="outt")
            nc.scalar.copy(outt[ql, :], ps_out[ql, :])
            nc.sync.dma_start(out=outd.ap()[q0:q0 + qlen, :], in_=outt[ql, :])

    nc.compile()
    return nc


_NC_CACHE = {}
LAST_RESULTS = None


def _get_nc():
    if "nc" not in _NC_CACHE:
        _NC_CACHE["nc"] = _build_program()
    return _NC_CACHE["nc"]


def host_prep(query, memory, ref_points, W_off, b_off, W_attn, b_attn,
              W_out, b_out):
    """Build the 8 per-core input maps (pure layout transforms)."""
    import ml_dtypes
    bf16 = ml_dtypes.bfloat16

    query = np.ascontiguousarray(query, dtype=np.float32)
    memory = np.ascontiguousarray(memory, dtype=np.float32)
    ref = np.asarray(ref_points, dtype=np.float32)
    W_off = np.asarray(W_off, dtype=np.float32)
    b_off = np.asarray(b_off, dtype=np.float32)
    W_attn = np.asarray(W_attn, dtype=np.float32)
    b_attn = np.asarray(b_attn, dtype=np.float32)
    assert np.all(b_off == 0.0) and np.all(b_attn == 0.0), \
        "nonzero offset/attn biases not folded on device"
    # W_off cols (h,l,p,xy) -> (l,h,p,xy)
    Woff_r = np.ascontiguousarray(
        W_off.reshape(C, NH, NL, NP, 2).transpose(0, 2, 1, 3, 4).reshape(C, 256))
    Wattn_r = np.ascontiguousarray(W_attn)  # cols already (h,l,p)
    Wout = np.ascontiguousarray(W_out, dtype=np.float32)
    bout = np.ascontiguousarray(np.asarray(b_out, dtype=np.float32).reshape(1, C))

    # ---- window geometry (all host-side, f32 to match device math) ----
    wh = np.array([[w, h] for h, w in SPATIAL], dtype=np.float32)
    refpix = ref.reshape(B, Q, NL, 2) * wh[None, None] - 0.5      # (x, y)
    vb = np.floor(refpix)
    lohi = np.array([[w - WIN, h - WIN] for h, w in SPATIAL], dtype=np.float32)
    xsc = np.clip(vb - 1.0, 0.0, lohi[None, None])                # window origin
    pxm_full = (refpix - xsc).astype(np.float32)                  # [B,Q,NL,2]
    xs = xsc[..., 0].astype(np.int64)
    ys = xsc[..., 1].astype(np.int64)

    # memory -> bf16 rows4 im2col  [B, R4_B, 4, C]
    mem_bf = memory.astype(bf16)
    rows4_parts = []
    base = 0
    for l, (h, w) in enumerate(SPATIAL):
        lvl = mem_bf[:, base:base + h * w].reshape(B, h, w, C)
        sw = np.lib.stride_tricks.sliding_window_view(lvl, WIN, axis=1)
        # sw: [B, h-3, w, C, 4] -> [B, (h-3)*w, 4, C]
        rows4_parts.append(np.ascontiguousarray(sw.transpose(0, 1, 2, 4, 3))
                           .reshape(B, R4_L[l], WIN * C))
        base += h * w
    rows4 = np.concatenate(rows4_parts, axis=1)                   # [B, R4_B, 4C]

    # gather slot index per (b, q, l)
    wl = np.array(W_L, dtype=np.int64)
    r4base = np.array(R4BASE, dtype=np.int64)
    slot = r4base[None, None] + ys * wl[None, None] + xs          # [B,Q,NL]

    in_maps = []
    for c in range(NCORES):
        bs = slice(BPC * c, BPC * (c + 1))
        qT = np.ascontiguousarray(query[bs].reshape(QS, C).T)     # [256, 600]
        mem4 = np.ascontiguousarray(rows4[bs].reshape(NR4, WIN * C))
        # slot index with per-batch offset, [QS, NL]
        sl = (slot[bs] + (np.arange(BPC) * R4_B)[:, None, None]).reshape(QS, NL)
        assert sl.max() < 32768
        # wrap for dma_gather: idx k = l*128 + q -> partition k%16, free
        # slot k//16, replicated over the 8 partition groups; per tile.
        idxw = np.zeros((128, NQT, 32), dtype=np.int16)
        for t in range(NQT):
            n = QT_SIZES[t]
            kflat = np.zeros((NL, 128), dtype=np.int16)
            kflat[:, :n] = sl[t * 128:t * 128 + n, :].T.astype(np.int16)
            wrapped = kflat.reshape(512 // 16, 16).T              # [16, 32]
            idxw[:, t, :] = np.tile(wrapped, (8, 1))
        pxm = np.ascontiguousarray(
            pxm_full[bs].reshape(QS, NL * 2).astype(np.float32))
        idxw = np.ascontiguousarray(idxw.reshape(128, NQT * 32))
        in_maps.append(dict(mem4=mem4, qT=qT, idxw=idxw, pxm=pxm, Woff=Woff_r,
                            Wattn=Wattn_r, Wout=Wout, bout=bout))
    return in_maps


def kernel(**inputs):
    global LAST_RESULTS
    from concourse.bass_utils import run_bass_kernel_spmd

    nc = _get_nc()
    in_maps = host_prep(**inputs)
    trace = bool(int(os.environ.get("KERNEL_TRACE", "0")))
    res = run_bass_kernel_spmd(nc, in_maps, core_ids=list(range(NCORES)),
                               trace=trace)
    LAST_RESULTS = res
    out = np.empty((B, Q, C), dtype=np.float32)
    for c in range(NCORES):
        out[BPC * c:BPC * (c + 1)] = res.results[c]["out"].reshape(BPC, Q, C)
    return out


# revision 13
# speedup vs baseline: 1.4279x; 1.4279x over previous
"""Trainium2 Bass kernel for DEIM multi-scale deformable attention (v2).

Strategy:
  - Data-parallel over batch: 16 batches -> 8 cores, 2 batches/core.
  - Host pre-packs memory as bf16 "rows4" im2col: rows4[(b,l,y,x)] holds
    pixel rows y..y+3 of column x (4 x 256 ch = 2KB), so one 4x4-pixel
    window x 256 ch is ONE contiguous 8KB dma_gather descriptor
    (order j=x-offset, r=y-offset, c).  Window origins, gather indices
    (pre-wrapped for dma_gather) and pxm = refpix - origin are all
    computed on the host - no on-device geometry or index bounce.
  - Device per query-tile (queries on partitions, 5 tiles of <=128):
    PE projections -> softmax -> bilinear hat factors ->
    stencil M[q,(h,j,r)] = sum_p attn*hatx*haty -> ACT broadcasts it to
    a bf16 meexp[q,(j,r,c)] -> DVE: one 2x-mode bf16 multiply with the
    gathered window + in-place bf16 add-tree over the 16 pixels ->
    f32 level sum -> PE output projection.
"""

import os
from contextlib import ExitStack

import numpy as np

# ---------------------------------------------------------------------------
# Problem constants (hardcoded per harness contract)
# ---------------------------------------------------------------------------
B, Q, C, NH, NP, NL = 16, 300, 256, 8, 4, 4
HD = C // NH
SPATIAL = ((80, 80), (40, 40), (20, 20), (30, 70))  # (h, w) per level
S = sum(h * w for h, w in SPATIAL)  # 10500
H_L = [h for h, w in SPATIAL]
W_L = [w for h, w in SPATIAL]

NCORES = 8
BPC = B // NCORES          # batches per core
QS = BPC * Q               # query slots per core (600)
QT_SIZES = [128, 128, 128, 128, QS - 4 * 128]  # [128,128,128,128,88]
NQT = len(QT_SIZES)
WIN = 4                    # window size (pixels per axis)
WELEM = WIN * WIN * C      # window elements (4096 bf16 = 8KB)

# rows4 geometry: per (batch, level) block of (h-3)*w start slots
R4_L = [(h - 3) * w for h, w in SPATIAL]          # [6160, 1480, 340, 1890]
R4_B = sum(R4_L)                                   # 9870 slots per batch
R4BASE = [0]
for v in R4_L[:-1]:
    R4BASE.append(R4BASE[-1] + v)                  # [0, 6160, 7640, 7980]
NR4 = BPC * R4_B                                   # 19740 slots per core


def _build_program():
    import concourse.bacc as bacc
    import concourse.bass as bass
    import concourse.tile as tile
    from concourse import mybir
    from concourse.masks import make_identity

    f32 = mybir.dt.float32
    bf16 = mybir.dt.bfloat16
    i16 = mybir.dt.int16

    nc = bacc.Bacc("TRN2", target_bir_lowering=False, debug=False,
                   num_devices=NCORES)

    AF = mybir.ActivationFunctionType
    OP = mybir.AluOpType

    def ap_of(t, off, pairs):
        """Manual access pattern on a tile/AP: offset in elements relative
        to t's own offset; pairs = [[step, count], ...] (partition first,
        rescaled to the tensor's per-partition stride; free steps in
        elements)."""
        a = t[:] if hasattr(t, "__getitem__") else t
        pairs = [list(p) for p in pairs]
        if a.space == bass.MemorySpace.SBUF:
            pairs[0][0] *= a.ap[0][0]
        return bass.AP(tensor=a.tensor, offset=a.offset + off, ap=pairs)

    # ------------------------------------------------------------------
    # DRAM I/O
    # ------------------------------------------------------------------
    mem4d = nc.dram_tensor("mem4", [NR4, WIN * C], bf16, kind="ExternalInput")
    qTd = nc.dram_tensor("qT", [C, QS], f32, kind="ExternalInput")
    idxd = nc.dram_tensor("idxw", [128, NQT * 32], i16, kind="ExternalInput")
    pxmd = nc.dram_tensor("pxm", [QS, 2 * NL], f32, kind="ExternalInput")
    woffd = nc.dram_tensor("Woff", [C, 256], f32, kind="ExternalInput")
    wattnd = nc.dram_tensor("Wattn", [C, NH * NL * NP], f32, kind="ExternalInput")
    woutd = nc.dram_tensor("Wout", [C, C], f32, kind="ExternalInput")
    boutd = nc.dram_tensor("bout", [1, C], f32, kind="ExternalInput")
    outd = nc.dram_tensor("out", [QS, C], f32, kind="ExternalOutput")

    with tile.TileContext(nc) as tc, ExitStack() as ctx:
        singles = ctx.enter_context(tc.tile_pool(name="singles", bufs=1))
        psum_mm = ctx.enter_context(tc.tile_pool(name="psum_mm", bufs=2, space="PSUM"))
        psum_tr = ctx.enter_context(tc.tile_pool(name="psum_tr", bufs=2, space="PSUM"))
        psum_o = ctx.enter_context(tc.tile_pool(name="psum_o", bufs=2, space="PSUM"))
        work = ctx.enter_context(tc.tile_pool(name="work", bufs=2))
        winp = ctx.enter_context(tc.tile_pool(name="winp", bufs=2))
        mep = ctx.enter_context(tc.tile_pool(name="mep", bufs=2))

        # ---------------- one-time constants ----------------
        sb_qT = singles.tile([128, 2, QS], f32)
        nc.sync.dma_start(out=sb_qT, in_=qTd.ap().rearrange("(k p) q -> p k q", p=128))
        sb_Woff = singles.tile([128, 2, 256], f32)
        nc.sync.dma_start(out=sb_Woff, in_=woffd.ap().rearrange("(k p) n -> p k n", p=128))
        sb_Wattn = singles.tile([128, 2, 128], f32)
        nc.sync.dma_start(out=sb_Wattn, in_=wattnd.ap().rearrange("(k p) n -> p k n", p=128))
        sb_Wout = singles.tile([128, 2, 256], f32)
        nc.sync.dma_start(out=sb_Wout, in_=woutd.ap().rearrange("(k p) n -> p k n", p=128))
        sb_bout = singles.tile([1, 256], f32)
        nc.sync.dma_start(out=sb_bout, in_=boutd.ap())
        sb_ones = singles.tile([1, 128], f32)
        nc.vector.memset(sb_ones, 1.0)
        ident = singles.tile([128, 128], f32)
        make_identity(nc, ident[:])
        # all gather indices, pre-wrapped by the host: [128, t, 32]
        sb_idx = singles.tile([128, NQT, 32], i16)
        nc.sync.dma_start(out=sb_idx, in_=idxd.ap())
        jneg = singles.tile([128, WIN], f32)
        for j in range(WIN):
            nc.vector.memset(jneg[:, j:j + 1], float(-j))

        # ---------------- per query-tile pipeline ----------------
        for it in range(NQT):
            q0 = it * 128
            qlen = QT_SIZES[it]
            ql = slice(0, qlen)

            # --- PE projections: offs [q, (l,h,p,xy)], logits [q, (h,l,p)]
            ps_off = psum_mm.tile([128, 256], f32, tag="ps_off")
            nc.tensor.matmul(ps_off[ql, :], lhsT=sb_qT[:, 0, q0:q0 + qlen],
                             rhs=sb_Woff[:, 0, :], start=True, stop=False)
            nc.tensor.matmul(ps_off[ql, :], lhsT=sb_qT[:, 1, q0:q0 + qlen],
                             rhs=sb_Woff[:, 1, :], start=False, stop=True)
            ps_log = psum_mm.tile([128, 128], f32, tag="ps_log")
            nc.tensor.matmul(ps_log[ql, :], lhsT=sb_qT[:, 0, q0:q0 + qlen],
                             rhs=sb_Wattn[:, 0, :], start=True, stop=False)
            nc.tensor.matmul(ps_log[ql, :], lhsT=sb_qT[:, 1, q0:q0 + qlen],
                             rhs=sb_Wattn[:, 1, :], start=False, stop=True)

            offs = work.tile([128, 256], f32, tag="offs")
            nc.scalar.copy(offs[ql, :], ps_off[ql, :])

            # --- softmax over (l,p) per h; logits cols are (h,l,p)
            elog = work.tile([128, 128], f32, tag="elog")
            nc.scalar.activation(elog[ql, :], ps_log[ql, :], AF.Exp)
            ssum = work.tile([128, NH], f32, tag="ssum")
            nc.vector.tensor_reduce(ssum[ql, :],
                                    elog[ql, :].rearrange("q (h s) -> q h s", h=NH),
                                    axis=mybir.AxisListType.X, op=OP.add)
            rinv = work.tile([128, NH], f32, tag="rinv")
            nc.vector.reciprocal(rinv[ql, :], ssum[ql, :])
            # attnR[q, (l,h,p)] = elog[q, h,l,p] * rinv[q, h]
            attnR = work.tile([128, 128], f32, tag="attnR")
            nc.vector.tensor_mul(
                attnR[ql, :],
                ap_of(elog, 0, [[1, qlen], [4, NL], [16, NH], [1, NP]]),
                ap_of(rinv, 0, [[1, qlen], [0, NL], [1, NH], [0, NP]]),
            )

            # --- window-relative positions pxm (host-computed) [q, (l,xy)]
            pxm = work.tile([128, 2 * NL], f32, tag="pxm")
            nc.sync.dma_start(out=pxm[ql, :], in_=pxmd.ap()[q0:q0 + qlen, :])

            # --- hats: U[q, (l,xy,hp)] = offs + pxm ;  H = relu(1 - |U - j|)
            uu = work.tile([128, NL, 2, 32], f32, tag="uu")
            nc.vector.tensor_add(
                ap_of(uu, 0, [[1, qlen], [64, NL], [32, 2], [1, 32]]),
                ap_of(offs, 0, [[1, qlen], [64, NL], [1, 2], [2, 32]]),
                ap_of(pxm, 0, [[1, qlen], [2, NL], [1, 2], [0, 32]]))
            hat = work.tile([128, WIN, NL, 2, 32], f32, tag="hat")
            for j in range(WIN):
                nc.scalar.activation(hat[ql, j, :, :, :],
                                     uu[ql, :, :, :], AF.Abs,
                                     bias=jneg[ql, j:j + 1])
            nc.scalar.activation(hat[ql, :, :, :, :], hat[ql, :, :, :, :],
                                 AF.Relu, bias=1.0, scale=-1.0)

            # AFJ[q, (l,h,p,j)] = attnR[q,(l,h,p)] * hatx[q,(j,l,hp)]
            afj = work.tile([128, NL, 8, NP, WIN], f32, tag="afj")
            nc.gpsimd.tensor_mul(
                afj[ql, :, :, :, :],
                ap_of(hat, 0, [[1, qlen], [64, NL], [1, 32], [256, WIN]]),
                ap_of(attnR, 0, [[1, qlen], [32, NL], [1, 32], [0, WIN]]))

            # --- one gather per tile: all 4 levels, idx k = l*128 + q
            win = winp.tile([128, NL, WELEM], bf16, tag="win")
            nc.gpsimd.dma_gather(
                out_ap=win[:, :, :],
                in_ap=ap_of(mem4d.ap(), 0, [[WIN * C, NR4 - (WIN - 1)], [1, WELEM]]),
                idxs_ap=sb_idx[:, it, :],
                num_idxs=512, num_idxs_reg=512,
                elem_size=WELEM, elem_step=WIN * C)

            # --- per level: stencil (GpSimd muls + DVE fused reduce),
            #     ACT broadcast, DVE multiply + unit-stride bf16 tree
            me3a = work.tile([128, NL, 128], f32, tag="me3a")
            res4 = work.tile([128, NL, 256], f32, tag="res4")
            for l in range(NL):
                # prod[q, (h,j,r), p] = afj[q,(l,h,p,j)] * haty[q,(r,l,hp)]
                prod = work.tile([128, 8 * WIN * WIN, NP], f32, tag="prod")
                for p in range(NP):
                    nc.gpsimd.tensor_mul(
                        ap_of(prod, p, [[1, qlen], [NP, 8 * WIN * WIN]]),
                        ap_of(afj, l * 128 + p * WIN,
                              [[1, qlen], [16, 8], [1, WIN], [0, WIN]]),
                        ap_of(hat, l * 64 + 32 + p,
                              [[1, qlen], [4, 8], [0, WIN], [256, WIN]]))
                # me3a[q, l, (j,r,h)] = sum_p prod[q, (h,j,r), p]
                nc.vector.tensor_reduce(
                    ap_of(me3a, l * 128, [[1, qlen], [1, 8], [32, WIN], [8, WIN]]),
                    prod[ql, :, :],
                    axis=mybir.AxisListType.X, op=OP.add)
                # meexp[q, (j,r,h)*32+rep] = me3a broadcast over 32 ch (ACT)
                meexp = mep.tile([128, WELEM], bf16, tag="meexp")
                nc.scalar.activation(
                    ap_of(meexp, 0, [[1, qlen], [32, 128], [1, 32]]),
                    ap_of(me3a, l * 128, [[1, qlen], [1, 128], [0, 32]]),
                    AF.Identity)
                # prodw = win_l * meexp in place (bf16 unit -> 2x mode)
                wl_ = ap_of(win, l * WELEM, [[1, qlen], [1, WELEM]])
                nc.vector.tensor_mul(wl_, wl_, meexp[ql, :])
                for half in (2048, 1024, 512):
                    nc.vector.tensor_add(
                        ap_of(win, l * WELEM, [[1, qlen], [1, half]]),
                        ap_of(win, l * WELEM, [[1, qlen], [1, half]]),
                        ap_of(win, l * WELEM + half, [[1, qlen], [1, half]]))
                nc.vector.tensor_add(
                    res4[ql, l, :],
                    ap_of(win, l * WELEM, [[1, qlen], [1, 256]]),
                    ap_of(win, l * WELEM + 256, [[1, qlen], [1, 256]]))

            # sum over levels (tree, f32)
            nc.vector.tensor_add(res4[ql, 0:2, :], res4[ql, 0:2, :], res4[ql, 2:4, :])
            res = work.tile([128, 256], f32, tag="res")
            nc.vector.tensor_add(res[ql, :], res4[ql, 0, :], res4[ql, 1, :])

            # --- output projection: out = res @ Wout + bout
            resT = work.tile([128, 2, 128], f32, tag="resT")
            for hh in range(2):
                ps_t = psum_tr.tile([128, 128], f32, tag="ps_t")
                nc.tensor.transpose(ps_t[:, ql], res[ql, 128 * hh:128 * (hh + 1)],
                                    ident[ql, ql])
                nc.scalar.copy(resT[:, hh, ql], ps_t[:, ql])
            ps_out = psum_o.tile([128, 256], f32, tag="ps_out")
            nc.tensor.matmul(ps_out[ql, :], lhsT=resT[:, 0, ql],
                             rhs=sb_Wout[:, 0, :], start=True, stop=False)
            nc.tensor.matmul(ps_out[ql, :], lhsT=resT[:, 1, ql],
                             rhs=sb_Wout[:, 1, :], start=False, stop=False)
            nc.tensor.matmul(ps_out[ql, :], lhsT=sb_ones[0:1, ql],
                             rhs=sb_bout[0:1, :], start=False, stop=True)
            outt = work.tile([128, 256], f32, tag="outt")
            nc.scalar.copy(outt[ql, :], ps_out[ql, :])
            nc.sync.dma_start(out=outd.ap()[q0:q0 + qlen, :], in_=outt[ql, :])

    nc.compile()
    return nc


_NC_CACHE = {}
LAST_RESULTS = None


def _get_nc():
    if "nc" not in _NC_CACHE:
        _NC_CACHE["nc"] = _build_program()
    return _NC_CACHE["nc"]


def host_prep(query, memory, ref_points, W_off, b_off, W_attn, b_attn,
              W_out, b_out):
    """Build the 8 per-core input maps (pure layout transforms)."""
    import ml_dtypes
    bf16 = ml_dtypes.bfloat16

    query = np.ascontiguousarray(query, dtype=np.float32)
    memory = np.ascontiguousarray(memory, dtype=np.float32)
    ref = np.asarray(ref_points, dtype=np.float32)
    W_off = np.asarray(W_off, dtype=np.float32)
    b_off = np.asarray(b_off, dtype=np.float32)
    W_attn = np.asarray(W_attn, dtype=np.float32)
    b_attn = np.asarray(b_attn, dtype=np.float32)
    assert np.all(b_off == 0.0) and np.all(b_attn == 0.0), \
        "nonzero offset/attn biases not folded on device"
    # W_off cols (h,l,p,xy) -> (l,h,p,xy)
    Woff_r = np.ascontiguousarray(
        W_off.reshape(C, NH, NL, NP, 2).transpose(0, 2, 1, 3, 4).reshape(C, 256))
    Wattn_r = np.ascontiguousarray(W_attn)  # cols already (h,l,p)
    Wout = np.ascontiguousarray(W_out, dtype=np.float32)
    bout = np.ascontiguousarray(np.asarray(b_out, dtype=np.float32).reshape(1, C))

    # ---- window geometry (all host-side, f32 to match device math) ----
    wh = np.array([[w, h] for h, w in SPATIAL], dtype=np.float32)
    refpix = ref.reshape(B, Q, NL, 2) * wh[None, None] - 0.5      # (x, y)
    vb = np.floor(refpix)
    lohi = np.array([[w - WIN, h - WIN] for h, w in SPATIAL], dtype=np.float32)
    xsc = np.clip(vb - 1.0, 0.0, lohi[None, None])                # window origin
    pxm_full = (refpix - xsc).astype(np.float32)                  # [B,Q,NL,2]
    xs = xsc[..., 0].astype(np.int64)
    ys = xsc[..., 1].astype(np.int64)

    # memory -> bf16 rows4 im2col  [B, R4_B, 4, C]
    mem_bf = memory.astype(bf16)
    rows4_parts = []
    base = 0
    for l, (h, w) in enumerate(SPATIAL):
        lvl = mem_bf[:, base:base + h * w].reshape(B, h, w, C)
        sw = np.lib.stride_tricks.sliding_window_view(lvl, WIN, axis=1)
        # sw: [B, h-3, w, C, 4] -> [B, (h-3)*w, 4, C]
        rows4_parts.append(np.ascontiguousarray(sw.transpose(0, 1, 2, 4, 3))
                           .reshape(B, R4_L[l], WIN * C))
        base += h * w
    rows4 = np.concatenate(rows4_parts, axis=1)                   # [B, R4_B, 4C]

    # gather slot index per (b, q, l)
    wl = np.array(W_L, dtype=np.int64)
    r4base = np.array(R4BASE, dtype=np.int64)
    slot = r4base[None, None] + ys * wl[None, None] + xs          # [B,Q,NL]

    in_maps = []
    for c in range(NCORES):
        bs = slice(BPC * c, BPC * (c + 1))
        qT = np.ascontiguousarray(query[bs].reshape(QS, C).T)     # [256, 600]
        mem4 = np.ascontiguousarray(rows4[bs].reshape(NR4, WIN * C))
        # slot index with per-batch offset, [QS, NL]
        sl = (slot[bs] + (np.arange(BPC) * R4_B)[:, None, None]).reshape(QS, NL)
        assert sl.max() < 32768
        # wrap for dma_gather: idx k = l*128 + q -> partition k%16, free
        # slot k//16, replicated over the 8 partition groups; per tile.
        idxw = np.zeros((128, NQT, 32), dtype=np.int16)
        for t in range(NQT):
            n = QT_SIZES[t]
            kflat = np.zeros((NL, 128), dtype=np.int16)
            kflat[:, :n] = sl[t * 128:t * 128 + n, :].T.astype(np.int16)
            wrapped = kflat.reshape(512 // 16, 16).T              # [16, 32]
            idxw[:, t, :] = np.tile(wrapped, (8, 1))
        pxm = np.ascontiguousarray(
            pxm_full[bs].reshape(QS, NL * 2).astype(np.float32))
        idxw = np.ascontiguousarray(idxw.reshape(128, NQT * 32))
        in_maps.append(dict(mem4=mem4, qT=qT, idxw=idxw, pxm=pxm, Woff=Woff_r,
                            Wattn=Wattn_r, Wout=Wout, bout=bout))
    return in_maps


def kernel(**inputs):
    global LAST_RESULTS
    from concourse.bass_utils import run_bass_kernel_spmd

    nc = _get_nc()
    in_maps = host_prep(**inputs)
    trace = bool(int(os.environ.get("KERNEL_TRACE", "0")))
    res = run_bass_kernel_spmd(nc, in_maps, core_ids=list(range(NCORES)),
                               trace=trace)
    LAST_RESULTS = res
    out = np.empty((B, Q, C), dtype=np.float32)
    for c in range(NCORES):
        out[BPC * c:BPC * (c + 1)] = res.results[c]["out"].reshape(BPC, Q, C)
    return out
